# revision 1
# baseline (speedup 1.0000x reference)
import numpy as np
import concourse.bass as bass
import concourse.bacc as bacc_mod
import concourse.mybir as mybir
from concourse import tile
from concourse.bass_utils import run_bass_kernel_spmd

B, I, K, O, D = 128, 1152, 8, 32, 16
NC = 8
IL = I // NC          # 144 capsules per core
OD = O * D            # 512
CH = 4                # i-chunk size
NCH = IL // CH        # 24 chunks
EPS = 1e-8
NROUTES = 3

F32 = mybir.dt.float32
BF16 = mybir.dt.bfloat16
ADD = mybir.AluOpType.add
MULT = mybir.AluOpType.mult
AF = mybir.ActivationFunctionType
AX = mybir.AxisListType


def _build():
    nc = bacc_mod.Bacc()
    wc_d = nc.declare_dram_parameter("wc", [K, IL, B + OD], BF16,
                                     isOutput=False)
    id_d = nc.declare_dram_parameter("ident", [B, B], BF16, isOutput=False)
    v_d = nc.declare_dram_parameter("vout", [B, OD], F32, isOutput=True)
    # collective bounce buffers (unique per route: avoids DMA reuse waits)
    ar_in = [nc.dram_tensor(f"ar_in{r}", [B, OD], F32) for r in range(3)]
    ar_out = [nc.dram_tensor(f"ar_out{r}", [B, OD], F32) for r in range(3)]

    with tile.TileContext(nc) as tc:
        with (
            tc.tile_pool(name="big", bufs=1) as big,
            tc.tile_pool(name="ld", bufs=2) as ld,
            tc.tile_pool(name="work", bufs=2) as work,
            tc.tile_pool(name="small", bufs=1) as small,
            tc.tile_pool(name="ps_a", bufs=4, space="PSUM") as ps_a,
            tc.tile_pool(name="ps_z", bufs=2, space="PSUM") as ps_z,
            tc.tile_pool(name="ps_s", bufs=1, space="PSUM") as ps_s,
        ):
            # persistent tiles
            xh = big.tile([B, IL * OD], BF16, tag="xh")      # 147KB/part
            ident = small.tile([B, B], BF16, tag="id")
            nc.sync.dma_start(out=ident[:], in_=id_d[:])
            zc = small.tile([B, 1], F32, tag="zc")
            nc.vector.memset(zc[:], 0.0)
            nc.const_aps.aps[(F32, 0.0)] = zc[:]
            zbuf = big.tile([B, IL * O], F32, tag="z")        # 18KB/part (z then e)
            cbuf = big.tile([B, IL * O], BF16, tag="c")       # 9KB/part
            u16 = small.tile([B, OD], BF16, tag="u16")
            vsum = small.tile([B, OD], F32, tag="vsum")       # v1+v2 accumulator
            sar = small.tile([B, OD], F32, tag="sar")         # allreduced s

            # ---------- phase A: x_hat + route-1 s (uniform c) ----------
            s_ps = ps_s.tile([B, OD], F32, tag="sps")
            BOD = B + OD
            for ic in range(NCH):
                w_t = ld.tile([K, CH * BOD], BF16, tag="wt")
                nc.gpsimd.dma_start(
                    out=w_t[:], in_=wc_d[:, ic * CH:(ic + 1) * CH, :])
                for j in range(CH):
                    i_g = ic * CH + j
                    xh_ps = ps_a.tile([B, OD], F32, tag="xhps")
                    nc.tensor.matmul(
                        xh_ps[:], w_t[:, j * BOD:j * BOD + B],
                        w_t[:, j * BOD + B:(j + 1) * BOD],
                        start=True, stop=True)
                    # evacuate to bf16 slab, alternate DVE/ACT
                    dst = xh[:, i_g * OD:(i_g + 1) * OD]
                    # DVE:ACT ~ 3:2 split matches their PSUM-copy rates
                    if i_g % 5 < 3:
                        nc.vector.tensor_copy(dst, xh_ps[:])
                    else:
                        nc.scalar.copy(dst, xh_ps[:])
                    # route-1 s accumulation: s1 = sum_i x_hat_i (uniform c)
                    nc.tensor.matmul(
                        s_ps[:], ident[:], dst,
                        start=(i_g == 0), stop=(i_g == IL - 1))

            def all_reduce_s(s_psum, scale, rno):
                s_loc = work.tile([B, OD], F32, tag="sq_t2")
                nc.scalar.mul(s_loc[:], s_psum[:], scale)
                nc.sync.dma_start(out=ar_in[rno][:], in_=s_loc[:])
                nc.gpsimd.collective_compute(
                    "AllReduce", ADD,
                    replica_groups=[list(range(NC))],
                    ins=[ar_in[rno][:]], outs=[ar_out[rno][:]])
                sar = small.tile([B, OD], F32, tag="sarX")
                nc.sync.dma_start(out=sar[:], in_=ar_out[rno][:])
                return sar

            def squash_to(vdst32, sar, roundno):
                # sar holds s [B, (o,d)]; compute v = s * q/((1+q)sqrt(q+eps))
                s3 = sar[:].rearrange("p (o d) -> p o d", o=O)
                t = work.tile([B, OD], F32, tag="sq_t2")
                nc.vector.tensor_mul(t[:], sar[:], sar[:])
                q = small.tile([B, O], F32, tag="qsq")
                nc.vector.tensor_reduce(
                    q[:], t[:].rearrange("p (o d) -> p o d", o=O),
                    axis=AX.X, op=ADD)
                qe = small.tile([B, O], F32, tag="qesq")
                nc.vector.tensor_scalar_add(qe[:], q[:], EPS)
                r = small.tile([B, O], F32, tag="rsq")
                nc.scalar.activation(r[:], qe[:], AF.Sqrt)
                t1 = small.tile([B, O], F32, tag="t1sq")
                nc.vector.scalar_tensor_tensor(
                    t1[:], q[:], 1.0, r[:], op0=ADD, op1=MULT)
                t2 = small.tile([B, O], F32, tag="t2sq")
                nc.vector.reciprocal(t2[:], t1[:])
                f = small.tile([B, O], F32, tag="fsq")
                nc.vector.tensor_mul(f[:], q[:], t2[:])
                fb = f[:].broadcast_to((B, O, D))
                nc.vector.tensor_mul(
                    vdst32[:].rearrange("p (o d) -> p o d", o=O), s3, fb)

            # ---------- route 1 ----------
            sar1 = all_reduce_s(s_ps, 1.0 / O, 0)
            squash_to(vsum, sar1, 1)
            nc.vector.tensor_copy(u16[:], vsum[:])   # u2 = v1 (bf16)

            # ---------- routes 2..3 ----------
            for rt in range(2, NROUTES + 1):
                last = rt == NROUTES
                # z = sum_d xhat * u  (DVE mult -> PE accum over d)
                ub = u16[:].rearrange("p (x o d) -> p x o d", x=1, o=O) \
                           .broadcast_to((B, CH, O, D))
                for ic in range(NCH):
                    y = work.tile([B, CH * OD], BF16, tag="y")
                    xs = xh[:, ic * CH * OD:(ic + 1) * CH * OD] \
                        .rearrange("p (i o d) -> p i o d", o=O, d=D)
                    nc.vector.tensor_mul(
                        y[:].rearrange("p (i o d) -> p i o d", o=O, d=D),
                        xs, ub)
                    z_ps = ps_z.tile([B, CH * O], F32, tag="zps")
                    yv = y[:].rearrange("p (i o d) -> p i o d", o=O, d=D)
                    for d in range(D):
                        nc.tensor.matmul(
                            z_ps[:], ident[:], yv[:, :, :, d],
                            start=(d == 0), stop=(d == D - 1))
                    # exp straight out of PSUM -> e (fp32, zbuf slab)
                    nc.scalar.activation(
                        zbuf[:, ic * CH * O:(ic + 1) * CH * O], z_ps[:],
                        AF.Exp)
                # softmax denom over o, then c = e * (1/sigma)
                sig = small.tile([B, IL], F32, tag="sig")
                nc.vector.tensor_reduce(
                    sig[:], zbuf[:].rearrange("p (i o) -> p i o", o=O),
                    axis=AX.X, op=ADD)
                rho = small.tile([B, IL], F32, tag="rho")
                nc.vector.reciprocal(rho[:], sig[:])
                rb = rho[:].rearrange("p (i x) -> p i x", x=1).broadcast_to((B, IL, O))
                nc.vector.tensor_mul(
                    cbuf[:].rearrange("p (i o) -> p i o", o=O),
                    zbuf[:].rearrange("p (i o) -> p i o", o=O), rb)
                # s = sum_i c * xhat  (DVE mult -> PE accum over i)
                s_ps2 = ps_s.tile([B, OD], F32, tag="sps")
                for ic in range(NCH):
                    sy = work.tile([B, CH * OD], BF16, tag="y")
                    cb = cbuf[:, ic * CH * O:(ic + 1) * CH * O] \
                        .rearrange("p (i o x) -> p i o x", o=O, x=1) \
                        .broadcast_to((B, CH, O, D))
                    xs = xh[:, ic * CH * OD:(ic + 1) * CH * OD] \
                        .rearrange("p (i o d) -> p i o d", o=O, d=D)
                    nc.vector.tensor_mul(
                        sy[:].rearrange("p (i o d) -> p i o d", o=O, d=D),
                        xs, cb)
                    for j in range(CH):
                        i_g = ic * CH + j
                        nc.tensor.matmul(
                            s_ps2[:], ident[:],
                            sy[:, j * OD:(j + 1) * OD],
                            start=(i_g == 0), stop=(i_g == IL - 1))
                sarR = all_reduce_s(s_ps2, 1.0, rt - 1)
                if last:
                    vout_t = small.tile([B, OD], F32, tag="vfin")
                    squash_to(vout_t, sarR, rt)
                    nc.sync.dma_start(out=v_d[:], in_=vout_t[:])
                else:
                    v2 = small.tile([B, OD], F32, tag="vfin")
                    squash_to(v2, sarR, rt)
                    nc.vector.tensor_add(vsum[:], vsum[:], v2[:])
                    nc.vector.tensor_copy(u16[:], vsum[:])  # u3 = v1+v2
    nc.compile()
    return nc


def _filter_bir(bir_json: bytes) -> bytes:
    """Drop same-ring WAW waits on DMAs (ring FIFO makes them redundant);
    the DIRECT2D descriptor only holds one wait command."""
    import json
    d = json.loads(bir_json)
    for fn in d.get("functions", []):
        for blk in fn.get("blocks", []):
            for inst in blk.get("instructions", []):
                if inst.get("opcode") != "DMACopy":
                    continue
                si = inst.get("sync_info") or {}
                waits = si.get("on_wait") or []
                if len(waits) <= 1:
                    continue
                ups = {u.get("ant_name") for u in (si.get("on_update") or [])}
                kept = [w for w in waits if w.get("ant_name") not in ups]
                if len(kept) < len(waits):
                    si["on_wait"] = kept
    return json.dumps(d).encode()


def _install_bir_filter():
    from concourse import bass2jax, bass_utils

    orig = bass_utils.compile_bir_kernel

    def patched(bir_json, tmpdir, neff_name="file.neff"):
        return orig(_filter_bir(bir_json), tmpdir, neff_name)

    bass2jax.compile_bir_kernel = patched


def _make_in_maps(x: np.ndarray, W: np.ndarray):
    ident = np.eye(B, dtype=np.float32)
    in_maps = []
    for c in range(NC):
        sl = slice(c * IL, (c + 1) * IL)
        xt = np.ascontiguousarray(
            x[:, sl, :].transpose(2, 1, 0)).astype(np.float32)  # [K, IL, B]
        wk = np.ascontiguousarray(
            W[sl].transpose(2, 0, 1, 3).reshape(K, IL, OD)).astype(np.float32)
        wc = np.concatenate([xt, wk], axis=2)  # [K, IL, B+OD]
        in_maps.append({"wc": _bf16(wc), "ident": _bf16(ident)})
    return in_maps


def kernel(x: np.ndarray, W: np.ndarray) -> np.ndarray:
    _install_bir_filter()
    nc = _build()
    in_maps = _make_in_maps(x, W)
    res = run_bass_kernel_spmd(nc, in_maps, list(range(NC)))
    v = np.asarray(res.results[0]["vout"], dtype=np.float32)
    return v.reshape(B, O, D)


def _bf16(a: np.ndarray):
    import jax.numpy as jnp
    return np.asarray(jnp.asarray(a, dtype=jnp.bfloat16))


if __name__ == "__main__":
    nc = _build()
    print("IR build OK")



# revision 18
# speedup vs baseline: 2119.6322x; 2119.6322x over previous
import numpy as np
import concourse.bass as bass
import concourse.bacc as bacc_mod
import concourse.mybir as mybir
from concourse import tile
from concourse.bass_utils import run_bass_kernel_spmd

B, I, K, O, D = 128, 1152, 8, 32, 16
NC = 8
IL = I // NC          # 144 capsules per core
IK = IL * K           # 1152 (i,k) rows per core
OD = O * D            # 512
BOD = B + OD          # combined x|W column block per capsule
CH = 8                # i's per route chunk
NCH = IL // CH        # 18 chunks
NW = IK // 128        # 9 (i,k)-row chunks of 128 for the fused s1 matmul
EPS = 1e-8

F32 = mybir.dt.float32
BF16 = mybir.dt.bfloat16
ADD = mybir.AluOpType.add
MULT = mybir.AluOpType.mult
AF = mybir.ActivationFunctionType
AX = mybir.AxisListType


NOCOLL = False
NODMA = False
def _build(repeats=1):
    nc = bacc_mod.Bacc()
    # xw: per-capsule x blocks (resident); ww: W blocks ((d,o) order), streamed
    xw_d = nc.declare_dram_parameter("xw", [K, IL * B], BF16, isOutput=False)
    ww_d = nc.declare_dram_parameter("ww", [K, IL, OD], BF16, isOutput=False)
    # xtw/wtw: (i,k)-row layout for the fused route-1 matmul
    xtw_d = nc.declare_dram_parameter("xtw", [IK, B], BF16, isOutput=False)
    wtw_d = nc.declare_dram_parameter("wtw", [IK, OD], BF16, isOutput=False)
    id_d = nc.declare_dram_parameter("ident", [B, B], BF16, isOutput=False)
    v_d = nc.declare_dram_parameter("vout", [B, OD], F32, isOutput=True)
    # collective bounce buffers, unique per (route, iteration parity)
    NAR = 3 * min(repeats, 2)
    ar_in = [nc.dram_tensor(f"ar_in{r}", [B, OD], BF16) for r in range(NAR)]
    ar_out = [nc.dram_tensor(f"ar_out{r}", [B, OD], BF16) for r in range(NAR)]

    with tile.TileContext(nc) as tc:
        with (
            tc.tile_pool(name="res", bufs=1) as res,
            tc.tile_pool(name="ld", bufs=3) as ld,
            tc.tile_pool(name="lds", bufs=2) as lds,
            tc.tile_pool(name="work", bufs=3) as work,
            tc.tile_pool(name="sm", bufs=3) as sm,
            tc.tile_pool(name="big", bufs=3) as big,
            tc.tile_pool(name="sq", bufs=2) as sq,
            tc.tile_pool(name="ps_a", bufs=2, space="PSUM") as ps_a,
            tc.tile_pool(name="ps_z", bufs=2, space="PSUM") as ps_z,
            tc.tile_pool(name="ps_s", bufs=2, space="PSUM") as ps_s,
        ):
            ident = res.tile([B, B], BF16, tag="id")
            nc.sync.dma_start(out=ident[:], in_=id_d[:])
            zc = res.tile([B, 1], F32, tag="zc")
            nc.vector.memset(zc[:], 0.0)
            nc.const_aps.aps[(F32, 0.0)] = zc[:]

            for rep in range(repeats):
                arb = 3 * (rep % min(repeats, 2))

                def all_reduce_s(s_sb, rno):
                    nc.sync.dma_start(out=ar_in[arb + rno][:], in_=s_sb[:])
                    if NOCOLL:
                        nc.gpsimd.dma_start(out=ar_out[arb + rno][:],
                                            in_=ar_in[arb + rno][:])
                    else:
                        nc.gpsimd.collective_compute(
                            "AllReduce", ADD,
                            replica_groups=[list(range(NC))],
                            ins=[ar_in[arb + rno][:]],
                            outs=[ar_out[arb + rno][:]])
                    sar = sm.tile([B, OD], BF16, tag="sar")
                    nc.sync.dma_start(out=sar[:], in_=ar_out[arb + rno][:])
                    return sar

                def squash_to(vdst32, sar):
                    # sar: [B, (d,o)] f32; v = s * q/((1+q)sqrt(q+eps))
                    t = sq.tile([B, OD], F32, tag="sq_t")
                    nc.vector.tensor_mul(t[:], sar[:], sar[:])
                    q = sq.tile([B, O], F32, tag="sq_q")
                    nc.vector.tensor_reduce(
                        q[:], t[:].rearrange("p (d o) -> p o d", d=D),
                        axis=AX.X, op=ADD)
                    qe = sq.tile([B, O], F32, tag="sq_qe")
                    nc.vector.tensor_scalar_add(qe[:], q[:], EPS)
                    r = sq.tile([B, O], F32, tag="sq_r")
                    nc.scalar.activation(r[:], qe[:], AF.Sqrt)
                    t1 = sq.tile([B, O], F32, tag="sq_t1")
                    nc.vector.scalar_tensor_tensor(
                        t1[:], q[:], 1.0, r[:], op0=ADD, op1=MULT)
                    t2 = sq.tile([B, O], F32, tag="sq_t2")
                    nc.vector.reciprocal(t2[:], t1[:])
                    f = sq.tile([B, O], F32, tag="sq_f")
                    nc.vector.tensor_mul(f[:], q[:], t2[:])
                    fb = f[:].rearrange("p (x o) -> p x o", x=1) \
                             .broadcast_to((B, D, O))
                    nc.vector.tensor_mul(
                        vdst32[:].rearrange("p (d o) -> p d o", d=D),
                        sar[:].rearrange("p (d o) -> p d o", d=D), fb)

                xw8 = res.tile([K, IL * B], BF16, tag="xw",
                               name=f"xw_{rep}")
                nc.sync.dma_start(out=xw8[:], in_=xw_d[:])

                # ---------- phase A: s1 = (1/O) sum_i x_hat (fused matmul) ---
                s1_ps = ps_s.tile([B, OD], F32, tag="sps")
                for c in range(NW):
                    xc = lds.tile([128, B], BF16, tag="xtw")
                    wc128 = lds.tile([128, OD], BF16, tag="wtw")
                    nc.gpsimd.dma_start(
                        out=xc[:], in_=xtw_d[c * 128:(c + 1) * 128, :])
                    nc.gpsimd.dma_start(
                        out=wc128[:], in_=wtw_d[c * 128:(c + 1) * 128, :])
                    nc.tensor.matmul(s1_ps[:], xc[:], wc128[:],
                                     start=(c == 0), stop=(c == NW - 1))
                s1_sb = sm.tile([B, OD], BF16, tag="s_loc")
                nc.scalar.mul(s1_sb[:], s1_ps[:], 1.0 / O)
                sar1 = all_reduce_s(s1_sb, 0)
                vsum = sm.tile([B, OD], F32, tag="vsum")
                squash_to(vsum, sar1)
                u16 = sm.tile([B, OD], BF16, tag="u16")
                nc.vector.tensor_copy(u16[:], vsum[:])

                def route(u, rno, last):
                    # 3-stage pipelined routing pass:
                    #  P0(ic): load W, xh matmuls -> PSUM -> evac to SBUF
                    #  P1(ic): y = xh*u (DVE), z = sum_d y (PE)
                    #  P2(ic): softmax (ACT/DVE), cy = c*xh (DVE), s += (PE)
                    # emitted as P0(ic), P1(ic-1), P2(ic-2) so every
                    # cross-engine dependency is at least one cycle old
                    s_ps = ps_s.tile([B, OD], F32, tag="sps")
                    qeng = [nc.sync, nc.gpsimd]
                    st = {}

                    def p0(ic):
                        w_t = ld.tile([K, CH * OD], BF16, tag="wt")
                        if not NODMA:
                            qeng[ic % 2].dma_start(
                                out=w_t[:],
                                in_=ww_d[:, ic * CH:(ic + 1) * CH, :])
                        xh_sb = big.tile([B, CH * OD], BF16, tag="xh")
                        xv = xh_sb[:].rearrange(
                            "p (d i o) -> p d i o", d=D, i=CH)
                        for pr in range(CH // 2):
                            xh_ps = ps_a.tile([B, 2 * OD], F32, tag="xhps")
                            for j2 in range(2):
                                j = pr * 2 + j2
                                i_g = ic * CH + j
                                nc.tensor.matmul(
                                    xh_ps[:, j2 * OD:(j2 + 1) * OD],
                                    xw8[:, i_g * B:(i_g + 1) * B],
                                    w_t[:, j * OD:(j + 1) * OD],
                                    start=True, stop=True)
                            dst = xv[:, :, pr * 2:pr * 2 + 2, :]
                            src_ = xh_ps[:].rearrange(
                                "p (j d o) -> p d j o", j=2, d=D)
                            nc.scalar.copy(dst, src_)
                        st[ic] = [xv]

                    def p1(ic):
                        xv, = st[ic]
                        y = work.tile([B, D * CH * O], BF16, tag="y")
                        ub = u[:].rearrange("p (d x o) -> p d x o",
                                            d=D, x=1) \
                                 .broadcast_to((B, D, CH, O))
                        nc.vector.tensor_mul(
                            y[:].rearrange("p (d i o) -> p d i o", d=D, i=CH),
                            xv, ub)
                        z_ps = ps_z.tile([B, CH * O], F32, tag="zps")
                        yv = y[:].rearrange("p (d i o) -> p d i o", d=D, i=CH)
                        for d in range(D):
                            nc.tensor.matmul(z_ps[:], ident[:], yv[:, d, :, :],
                                             start=(d == 0),
                                             stop=(d == D - 1))
                        st[ic].append(z_ps)

                    def p2(ic):
                        xv, z_ps = st.pop(ic)
                        e_t = sm.tile([B, CH * O], BF16, tag="e")
                        nc.scalar.activation(e_t[:], z_ps[:], AF.Exp)
                        sg = sm.tile([B, CH], F32, tag="sg")
                        nc.vector.tensor_reduce(
                            sg[:], e_t[:].rearrange("p (i o) -> p i o", i=CH),
                            axis=AX.X, op=ADD)
                        rho = sm.tile([B, CH], F32, tag="rho")
                        nc.vector.reciprocal(rho[:], sg[:])
                        c_t = sm.tile([B, CH * O], BF16, tag="c")
                        rb = rho[:].rearrange("p (i x) -> p i x", x=1) \
                                   .broadcast_to((B, CH, O))
                        nc.vector.tensor_mul(
                            c_t[:].rearrange("p (i o) -> p i o", i=CH),
                            e_t[:].rearrange("p (i o) -> p i o", i=CH), rb)
                        cy = work.tile([B, D * CH * O], BF16, tag="cy")
                        cb = c_t[:].rearrange("p (x i o) -> p x i o",
                                              x=1, i=CH) \
                                   .broadcast_to((B, D, CH, O))
                        nc.vector.tensor_mul(
                            cy[:].rearrange("p (d i o) -> p d i o", d=D, i=CH),
                            xv, cb)
                        cyv = cy[:].rearrange("p (d i o) -> p d i o",
                                              d=D, i=CH)
                        for j in range(CH):
                            i_g = ic * CH + j
                            nc.tensor.matmul(s_ps[:], ident[:],
                                             cyv[:, :, j, :],
                                             start=(i_g == 0),
                                             stop=(i_g == IL - 1))

                    for ic in range(NCH + 2):
                        if ic < NCH:
                            p0(ic)
                        if 1 <= ic < NCH + 1:
                            p1(ic - 1)
                        if ic >= 2:
                            p2(ic - 2)
                    s_sb = sm.tile([B, OD], BF16, tag="s_loc")
                    nc.scalar.copy(s_sb[:], s_ps[:])
                    sar = all_reduce_s(s_sb, rno)
                    if last:
                        v3 = sm.tile([B, OD], F32, tag="v3")
                        squash_to(v3, sar)
                        nc.sync.dma_start(out=v_d[:], in_=v3[:])
                    else:
                        v2 = sm.tile([B, OD], F32, tag="v2")
                        squash_to(v2, sar)
                        nc.vector.tensor_add(vsum[:], vsum[:], v2[:])
                        u16b = sm.tile([B, OD], BF16, tag="u16b")
                        nc.vector.tensor_copy(u16b[:], vsum[:])
                        return u16b

                u16b = route(u16, 1, last=False)
                route(u16b, 2, last=True)
    nc.compile()
    return nc


def _filter_bir(bir_json: bytes) -> bytes:
    """Drop same-ring WAW waits on DMAs (ring FIFO makes them redundant);
    the DIRECT2D descriptor only holds one wait command."""
    import json
    d = json.loads(bir_json)
    for fn in d.get("functions", []):
        for blk in fn.get("blocks", []):
            for inst in blk.get("instructions", []):
                if inst.get("opcode") != "DMACopy":
                    continue
                si = inst.get("sync_info") or {}
                waits = si.get("on_wait") or []
                if len(waits) <= 1:
                    continue
                ups = {u.get("ant_name") for u in (si.get("on_update") or [])}
                kept = [w for w in waits if w.get("ant_name") not in ups]
                if len(kept) < len(waits):
                    si["on_wait"] = kept
    return json.dumps(d).encode()


def _install_bir_filter():
    from concourse import bass2jax, bass_utils

    orig = bass_utils.compile_bir_kernel

    def patched(bir_json, tmpdir, neff_name="file.neff"):
        return orig(_filter_bir(bir_json), tmpdir, neff_name)

    bass2jax.compile_bir_kernel = patched


def _bf16(a: np.ndarray):
    import jax.numpy as jnp
    return np.asarray(jnp.asarray(a, dtype=jnp.bfloat16))


def _make_in_maps(x: np.ndarray, W: np.ndarray):
    ident = np.eye(B, dtype=np.float32)
    in_maps = []
    for c in range(NC):
        sl = slice(c * IL, (c + 1) * IL)
        xw = np.ascontiguousarray(
            x[:, sl, :].transpose(2, 1, 0).reshape(K, IL * B)
        ).astype(np.float32)
        ww = np.ascontiguousarray(
            W[sl].transpose(2, 0, 3, 1).reshape(K, IL, OD)).astype(np.float32)
        # fused-s1 layout: rows (i,k)
        xtw = np.ascontiguousarray(
            x[:, sl, :].transpose(1, 2, 0).reshape(IK, B)).astype(np.float32)
        wtw = np.ascontiguousarray(
            W[sl].transpose(0, 2, 3, 1).reshape(IK, OD)).astype(np.float32)
        in_maps.append({"xw": _bf16(xw), "ww": _bf16(ww), "xtw": _bf16(xtw),
                        "wtw": _bf16(wtw), "ident": _bf16(ident)})
    return in_maps


def kernel(x: np.ndarray, W: np.ndarray) -> np.ndarray:
    _install_bir_filter()
    nc = _build()
    in_maps = _make_in_maps(x, W)
    res = run_bass_kernel_spmd(nc, in_maps, list(range(NC)))
    v = np.asarray(res.results[0]["vout"], dtype=np.float32)
    # device layout is [B, (d,o)] -> [B, O, D]
    return v.reshape(B, D, O).transpose(0, 2, 1)


if __name__ == "__main__":
    nc = _build()
    print("IR build OK")
